# revision 16
# baseline (speedup 1.0000x reference)
"""MoE (DeepSeek-style) routed+shared expert forward on 8 TRN2 NeuronCores.

Strategy (expert-parallel, host-side dispatch):
  - Host computes the gate (softmax + top-2) in float64 and gathers each
    expert's routed tokens (this is the "all-to-all by routing index" --
    with full inputs on the host, the host does the dispatch).
  - Core e processes expert e's routed tokens (padded to a uniform
    capacity C) through the SwiGLU FFN, plus a 1/8 slice of all tokens
    through the replicated shared-expert MLP.
  - All activations/weights are fed transposed (features on SBUF
    partitions, tokens on the free dim) so the w1/w3 -> swiglu -> w2
    chain needs no on-chip transposes.
  - Matmuls use float32r (full-rate fp32 on the PE array).
  - Host scatters expert outputs back by routing index, scales by the
    gate weights, and adds the shared-expert output.
"""

import sys

if "/opt/trn_rl_repo" not in sys.path:
    sys.path.insert(0, "/opt/trn_rl_repo")

import ml_dtypes
import numpy as np

import concourse.bass as bass
import concourse.tile as tile
from concourse import bacc, mybir
from concourse import bass_utils

B, S, DIM = 4, 2048, 1024
T = B * S
INTER = 1024
E = 8
TOPK = 2
ROUTE_SCALE = 1.0
SHARED_INTER = 2048
N_CORES = 8
TOKS_SHARED = T // N_CORES  # shared-expert tokens per core
BLK = 512

F32 = mybir.dt.float32
F32R = mybir.dt.float32r
BF16 = mybir.dt.bfloat16
SILU = mybir.ActivationFunctionType.Silu
IDENT = mybir.ActivationFunctionType.Identity

_program_cache = {}


def _blocks(total):
    out = []
    o = 0
    while o < total:
        n = min(BLK, total - o)
        out.append((o, n))
        o += n
    return out


def build_program(C):
    """Build the per-core SPMD Bass program for routed capacity C.

    Phase 1 (routed expert): fp32r matmuls, w1/w3/w2 resident in SBUF.
    Phase 2 (shared expert): bf16 matmuls, ws1/ws3/ws2 resident in SBUF,
    tokens processed in two halves of 512. Each weight/activation chunk
    is a separate tile so matmuls depend only on the chunk they read;
    DMA issue order matches PE consumption order.
    """
    nc = bacc.Bacc("TRN2", target_bir_lowering=False, debug=False,
                   num_devices=N_CORES)

    def din(name, shape, dt=F32):
        return nc.dram_tensor(name, shape, dt, kind="ExternalInput").ap()

    def dout(name, shape):
        return nc.dram_tensor(name, shape, F32, kind="ExternalOutput").ap()

    xe = din("xe", (DIM, C), F32R)            # routed tokens, transposed
    xs = din("xs", (DIM, TOKS_SHARED), BF16)  # shared-token slice, transposed
    w1t = din("w1t", (DIM, INTER), F32R)      # w1[e].T
    w3t = din("w3t", (DIM, INTER), F32R)
    w2t = din("w2t", (INTER, DIM), F32R)      # w2[e].T
    ws1t = din("ws1t", (DIM, SHARED_INTER), BF16)
    ws3t = din("ws3t", (DIM, SHARED_INTER), BF16)
    ws2t = din("ws2t", (SHARED_INTER, DIM), BF16)
    b1 = din("b1", (INTER,))
    b3 = din("b3", (INTER,))
    b2 = din("b2", (DIM,))
    bs1 = din("bs1", (SHARED_INTER,))
    bs3 = din("bs3", (SHARED_INTER,))
    bs2 = din("bs2", (DIM,))
    ye = dout("ye", (DIM, C))
    ys = dout("ys", (DIM, TOKS_SHARED))

    ND = DIM // 128           # 8 k-tiles over DIM
    NI = INTER // 128         # 8 tiles over INTER
    NS = SHARED_INTER // 128  # 16 tiles over SHARED_INTER

    xe_r = xe.rearrange("(dk p) c -> p dk c", p=128)
    xs_r = xs.rearrange("(dk p) c -> p dk c", p=128)
    w1_r = w1t.rearrange("(dk p) i -> p dk i", p=128)
    w3_r = w3t.rearrange("(dk p) i -> p dk i", p=128)
    w2_r = w2t.rearrange("(mi p) d -> p mi d", p=128)
    ws1_r = ws1t.rearrange("(dk p) i -> p dk i", p=128)
    ws3_r = ws3t.rearrange("(dk p) i -> p dk i", p=128)
    ws2_r = ws2t.rearrange("(mi p) d -> p mi d", p=128)
    ye_r = ye.rearrange("(md p) c -> p md c", p=128)
    ys_r = ys.rearrange("(md p) c -> p md c", p=128)

    with tile.TileContext(nc) as tc:
        with tc.tile_pool(name="bias", bufs=1) as bpool, \
             tc.tile_pool(name="tmp", bufs=2) as tpool, \
             tc.tile_pool(name="yout", bufs=3) as ypool, \
             tc.tile_pool(name="ps", bufs=2, space="PSUM") as pspool:
            b1_sb = bpool.tile([128, NI], F32, tag="b1")
            nc.sync.dma_start(b1_sb[:], b1.rearrange("(mi p) -> p mi", p=128))
            b3_sb = bpool.tile([128, NI], F32, tag="b3")
            nc.sync.dma_start(b3_sb[:], b3.rearrange("(mi p) -> p mi", p=128))
            b2_sb = bpool.tile([128, ND], F32, tag="b2")
            nc.sync.dma_start(b2_sb[:], b2.rearrange("(md p) -> p md", p=128))
            bs1_sb = bpool.tile([128, NS], F32, tag="bs1")
            nc.sync.dma_start(bs1_sb[:], bs1.rearrange("(mi p) -> p mi", p=128))
            bs3_sb = bpool.tile([128, NS], F32, tag="bs3")
            nc.sync.dma_start(bs3_sb[:], bs3.rearrange("(mi p) -> p mi", p=128))
            bs2_sb = bpool.tile([128, ND], F32, tag="bs2")
            nc.sync.dma_start(bs2_sb[:], bs2.rearrange("(md p) -> p md", p=128))

            blocks = _blocks(C)
            nb = len(blocks)

            def load_xb(xpool, off, n):
                xb = []
                for dk in range(ND):
                    t = xpool.tile([128, n], F32R, tag=f"xb{dk}",
                                   name=f"xb{dk}", padded_shape=[128, BLK])
                    h = n // 2 if n >= 2 else n
                    nc.sync.dma_start(t[:, 0:h], xe_r[:, dk, off:off + h])
                    if h < n:
                        nc.sync.dma_start(t[:, h:n], xe_r[:, dk, off + h:off + n])
                    xb.append(t)
                return xb

            def mi_stage(w1_sb, w3_sb, xb, hb, n, width, nk, t1b, t3b, hoff=0):
                nm = width // 128
                for mi in range(nm):
                    ps1 = pspool.tile([128, n], F32, tag="ps1",
                                      padded_shape=[128, BLK])
                    ps3 = pspool.tile([128, n], F32, tag="ps3",
                                      padded_shape=[128, BLK])
                    for dk in range(nk):
                        nc.tensor.matmul(
                            ps1[:], w1_sb[dk][:, mi * 128:(mi + 1) * 128],
                            xb[dk][:, hoff:hoff + n],
                            start=(dk == 0), stop=(dk == nk - 1))
                    for dk in range(nk):
                        nc.tensor.matmul(
                            ps3[:], w3_sb[dk][:, mi * 128:(mi + 1) * 128],
                            xb[dk][:, hoff:hoff + n],
                            start=(dk == 0), stop=(dk == nk - 1))
                    hdt = hb[mi].dtype
                    tdt = BF16 if hdt == BF16 else F32
                    t1 = tpool.tile([128, n], tdt, tag=f"t1{tdt}",
                                    name="t1", padded_shape=[128, BLK])
                    nc.scalar.activation(t1[:], ps1[:], SILU,
                                         bias=t1b[:, mi:mi + 1])
                    t3 = tpool.tile([128, n], tdt, tag=f"t3{tdt}",
                                    name="t3", padded_shape=[128, BLK])
                    nc.scalar.activation(t3[:], ps3[:], IDENT,
                                         bias=t3b[:, mi:mi + 1])
                    nc.vector.tensor_mul(hb[mi][:], t1[:], t3[:])

            def md_stage(w2_sb, hb, out_r, off, n, nmi, b2b):
                for md in range(ND):
                    psy = pspool.tile([128, n], F32, tag="psy",
                                      padded_shape=[128, BLK])
                    for mi in range(nmi):
                        nc.tensor.matmul(
                            psy[:], w2_sb[mi][:, md * 128:(md + 1) * 128],
                            hb[mi][:],
                            start=(mi == 0), stop=(mi == nmi - 1))
                    yt = ypool.tile([128, n], F32, tag="yt",
                                    name="yt", padded_shape=[128, BLK])
                    nc.scalar.activation(yt[:], psy[:], IDENT,
                                         bias=b2b[:, md:md + 1])
                    nc.sync.dma_start(out_r[:, md, off:off + n], yt[:])

            # ---------- Phase 1: routed expert (fp32r, weights resident) ----
            from contextlib import ExitStack
            es1 = ExitStack()   # xb/hb/w2 pools: released before phase 2 bulk
            es2 = ExitStack()   # w1/w3 pool: released before last w2 stage
            esA = ExitStack()   # xs/ws1 pool: lives until kernel end
            xpool = es1.enter_context(tc.tile_pool(name="xbp", bufs=2))
            hpool = es1.enter_context(tc.tile_pool(name="hbp", bufs=1))
            wpool2 = es1.enter_context(tc.tile_pool(name="wp2", bufs=1))
            wpool13 = es2.enter_context(tc.tile_pool(name="wp13", bufs=1))

            # First block's activations + the first quarter of each w1
            # chunk go out first, so the first matmuls are gated on ~1MB;
            # the rest streams behind the running PE.
            xb0 = load_xb(xpool, 0, blocks[0][1])
            w1_sb, w3_sb, w2_sb = [], [], []
            for dk in range(ND):
                t = wpool13.tile([128, INTER], F32R, tag=f"w1_{dk}")
                nc.sync.dma_start(t[:, 0:256], w1_r[:, dk, 0:256])
                w1_sb.append(t)
            for q in range(1, 4):
                for dk in range(ND):
                    nc.sync.dma_start(
                        w1_sb[dk][:, q * 256:(q + 1) * 256],
                        w1_r[:, dk, q * 256:(q + 1) * 256])
            for dk in range(ND):
                t = wpool13.tile([128, INTER], F32R, tag=f"w3_{dk}")
                nc.sync.dma_start(t[:, 0:512], w3_r[:, dk, 0:512])
                nc.sync.dma_start(t[:, 512:1024], w3_r[:, dk, 512:1024])
                w3_sb.append(t)
            for mi in range(NI):
                t = wpool2.tile([128, DIM], F32R, tag=f"w2_{mi}")
                nc.sync.dma_start(t[:, 0:512], w2_r[:, mi, 0:512])
                nc.sync.dma_start(t[:, 512:1024], w2_r[:, mi, 512:1024])
                w2_sb.append(t)

            last = None
            for bi, (off, n) in enumerate(blocks):
                xb = xb0 if bi == 0 else load_xb(xpool, off, n)
                hb = [hpool.tile([128, n], F32R, tag=f"hb{mi}",
                                 name=f"hb{mi}", padded_shape=[128, BLK])
                      for mi in range(NI)]
                mi_stage(w1_sb, w3_sb, xb, hb, n, INTER, ND, b1_sb, b3_sb)
                if bi < nb - 1:
                    md_stage(w2_sb, hb, ye_r, off, n, NI, b2_sb)
                else:
                    last = (hb, off, n)

            # w1/w3 released: xs + ws1 stream in during the last w2 stage.
            es2.close()
            wspoolA = esA.enter_context(tc.tile_pool(name="wshA", bufs=1, side="right"))
            xs_sb, ws1_sb = [], []
            for dk in range(ND):
                t = wspoolA.tile([128, TOKS_SHARED], BF16, tag=f"xs{dk}")
                nc.sync.dma_start(t[:], xs_r[:, dk, :])
                xs_sb.append(t)
                t = wspoolA.tile([128, SHARED_INTER], BF16, tag=f"ws1_{dk}")
                nc.sync.dma_start(t[:], ws1_r[:, dk, :])
                ws1_sb.append(t)
            (hb, off, n) = last
            md_stage(w2_sb, hb, ye_r, off, n, NI, b2_sb)
            es1.close()

            # ---------- Phase 2: shared expert (bf16, weights resident) -----
            with tc.tile_pool(name="wshB", bufs=1, side="right") as wspoolB, \
                 tc.tile_pool(name="hsp", bufs=1, side="right") as hspool:
                ws3_sb, ws2_sb = [], []
                for dk in range(ND):
                    t = wspoolB.tile([128, SHARED_INTER], BF16,
                                     tag=f"ws3_{dk}")
                    nc.sync.dma_start(t[:, 0:1024], ws3_r[:, dk, 0:1024])
                    nc.sync.dma_start(t[:, 1024:2048], ws3_r[:, dk, 1024:2048])
                    ws3_sb.append(t)
                for mi in range(NS):
                    t = wspoolB.tile([128, DIM], BF16, tag=f"ws2_{mi}")
                    nc.sync.dma_start(t[:], ws2_r[:, mi, :])
                    ws2_sb.append(t)

                for (off, n) in _blocks(TOKS_SHARED):
                    hs = [hspool.tile([128, n], BF16, tag=f"hs{mi}",
                                      name=f"hs{mi}", padded_shape=[128, BLK])
                          for mi in range(NS)]
                    mi_stage(ws1_sb, ws3_sb, xs_sb, hs, n, SHARED_INTER, ND,
                             bs1_sb, bs3_sb, hoff=off)
                    md_stage(ws2_sb, hs, ys_r, off, n, NS, bs2_sb)
            esA.close()

    nc.compile()
    return nc


def _gate_host(xt, gate_w, gate_b):
    """Softmax gate + top-2 routing, computed in float64 on the host."""
    logits = xt.astype(np.float64) @ gate_w.astype(np.float64).T \
        + gate_b.astype(np.float64)
    m = logits.max(axis=-1, keepdims=True)
    p = np.exp(logits - m)
    scores = p / p.sum(axis=-1, keepdims=True)
    order = np.argsort(-scores, axis=1, kind="stable")
    top_i = order[:, :TOPK]
    top_w = (np.take_along_axis(scores, top_i, axis=1)
             * ROUTE_SCALE).astype(np.float32)
    return top_i, top_w


def run(inputs, trace=False):
    x = np.ascontiguousarray(np.asarray(inputs["x"], dtype=np.float32))
    gate_w = np.asarray(inputs["gate_w"], dtype=np.float32)
    gate_b = np.asarray(inputs["gate_b"], dtype=np.float32)
    w1 = np.asarray(inputs["w1"], dtype=np.float32)
    b1 = np.asarray(inputs["b1"], dtype=np.float32)
    w3 = np.asarray(inputs["w3"], dtype=np.float32)
    b3 = np.asarray(inputs["b3"], dtype=np.float32)
    w2 = np.asarray(inputs["w2"], dtype=np.float32)
    b2 = np.asarray(inputs["b2"], dtype=np.float32)
    ws1 = np.asarray(inputs["ws1"], dtype=np.float32)
    bs1 = np.asarray(inputs["bs1"], dtype=np.float32)
    ws3 = np.asarray(inputs["ws3"], dtype=np.float32)
    bs3 = np.asarray(inputs["bs3"], dtype=np.float32)
    ws2 = np.asarray(inputs["ws2"], dtype=np.float32)
    bs2 = np.asarray(inputs["bs2"], dtype=np.float32)

    xt = x.reshape(T, DIM)
    top_i, top_w = _gate_host(xt, gate_w, gate_b)

    # Dispatch: token lists + gate weights per expert.
    idx, wgt = [], []
    for e in range(E):
        toks = np.nonzero((top_i == e).any(axis=1))[0]
        idx.append(toks)
        slot = (top_i[toks] == e)            # [n_e, TOPK], exactly one True/row
        wgt.append(top_w[toks][slot])

    cmax = max(len(i) for i in idx)
    C = max(256, -(-cmax // 256) * 256)

    bf16 = ml_dtypes.bfloat16
    ws1t = np.ascontiguousarray(ws1.T).astype(bf16)
    ws3t = np.ascontiguousarray(ws3.T).astype(bf16)
    ws2t = np.ascontiguousarray(ws2.T).astype(bf16)
    xt_bf = xt.astype(bf16)

    in_maps = []
    for e in range(E):
        xe = np.zeros((DIM, C), np.float32)
        xe[:, :len(idx[e])] = xt[idx[e]].T
        sl = slice(TOKS_SHARED * e, TOKS_SHARED * (e + 1))
        in_maps.append({
            "xe": xe,
            "xs": np.ascontiguousarray(xt_bf[sl].T),
            "w1t": np.ascontiguousarray(w1[e].T),
            "w3t": np.ascontiguousarray(w3[e].T),
            "w2t": np.ascontiguousarray(w2[e].T),
            "ws1t": ws1t, "ws3t": ws3t, "ws2t": ws2t,
            "b1": b1[e], "b3": b3[e], "b2": b2[e],
            "bs1": bs1, "bs3": bs3, "bs2": bs2,
        })

    if C not in _program_cache:
        _program_cache[C] = build_program(C)
    nc = _program_cache[C]

    res = bass_utils.run_bass_kernel_spmd(
        nc, in_maps, core_ids=list(range(N_CORES)), trace=trace)

    y = np.empty((T, DIM), np.float32)
    for e in range(E):
        sl = slice(TOKS_SHARED * e, TOKS_SHARED * (e + 1))
        y[sl] = res.results[e]["ys"].T
    for e in range(E):
        ye = res.results[e]["ye"]
        y[idx[e]] += ye[:, :len(idx[e])].T * wgt[e][:, None]
    return y.reshape(B, S, DIM), res


def kernel(**inputs) -> np.ndarray:
    out, _ = run(inputs, trace=False)
    return out


# revision 17
# speedup vs baseline: 1.0532x; 1.0532x over previous
"""MoE (DeepSeek-style) routed+shared expert forward on 8 TRN2 NeuronCores.

Strategy (expert-parallel, host-side dispatch):
  - Host computes the gate (softmax + top-2) in float64 and gathers each
    expert's routed tokens (this is the "all-to-all by routing index" --
    with full inputs on the host, the host does the dispatch).
  - Core e processes expert e's routed tokens (padded to a uniform
    capacity C) through the SwiGLU FFN, plus a 1/8 slice of all tokens
    through the replicated shared-expert MLP.
  - All activations/weights are fed transposed (features on SBUF
    partitions, tokens on the free dim) so the w1/w3 -> swiglu -> w2
    chain needs no on-chip transposes.
  - Matmuls use float32r (full-rate fp32 on the PE array).
  - Host scatters expert outputs back by routing index, scales by the
    gate weights, and adds the shared-expert output.
"""

import sys

if "/opt/trn_rl_repo" not in sys.path:
    sys.path.insert(0, "/opt/trn_rl_repo")

import ml_dtypes
import numpy as np

import concourse.bass as bass
import concourse.tile as tile
from concourse import bacc, mybir
from concourse import bass_utils

B, S, DIM = 4, 2048, 1024
T = B * S
INTER = 1024
E = 8
TOPK = 2
ROUTE_SCALE = 1.0
SHARED_INTER = 2048
N_CORES = 8
TOKS_SHARED = T // N_CORES  # shared-expert tokens per core
BLK = 512

F32 = mybir.dt.float32
F32R = mybir.dt.float32r
BF16 = mybir.dt.bfloat16
SILU = mybir.ActivationFunctionType.Silu
IDENT = mybir.ActivationFunctionType.Identity

_program_cache = {}


def _blocks(total):
    out = []
    o = 0
    while o < total:
        n = min(BLK, total - o)
        out.append((o, n))
        o += n
    return out


def build_program(C):
    """Build the per-core SPMD Bass program for routed capacity C.

    Phase 1 (routed expert): fp32r matmuls, w1/w3/w2 resident in SBUF.
    Phase 2 (shared expert): bf16 matmuls, ws1/ws3/ws2 resident in SBUF,
    tokens processed in two halves of 512. Each weight/activation chunk
    is a separate tile so matmuls depend only on the chunk they read;
    DMA issue order matches PE consumption order.
    """
    nc = bacc.Bacc("TRN2", target_bir_lowering=False, debug=False,
                   num_devices=N_CORES)

    def din(name, shape, dt=F32):
        return nc.dram_tensor(name, shape, dt, kind="ExternalInput").ap()

    def dout(name, shape):
        return nc.dram_tensor(name, shape, F32, kind="ExternalOutput").ap()

    xe = din("xe", (DIM, C), F32R)            # routed tokens, transposed
    xs = din("xs", (DIM, TOKS_SHARED), BF16)  # shared-token slice, transposed
    w1t = din("w1t", (DIM, INTER), F32R)      # w1[e].T
    w3t = din("w3t", (DIM, INTER), F32R)
    w2t = din("w2t", (INTER, DIM), F32R)      # w2[e].T
    ws1t = din("ws1t", (DIM, SHARED_INTER), BF16)
    ws3t = din("ws3t", (DIM, SHARED_INTER), BF16)
    ws2t = din("ws2t", (SHARED_INTER, DIM), BF16)
    b1 = din("b1", (INTER,))
    b3 = din("b3", (INTER,))
    b2 = din("b2", (DIM,))
    bs1 = din("bs1", (SHARED_INTER,))
    bs3 = din("bs3", (SHARED_INTER,))
    bs2 = din("bs2", (DIM,))
    ye = dout("ye", (DIM, C))
    ys = dout("ys", (DIM, TOKS_SHARED))

    ND = DIM // 128           # 8 k-tiles over DIM
    NI = INTER // 128         # 8 tiles over INTER
    NS = SHARED_INTER // 128  # 16 tiles over SHARED_INTER

    xe_r = xe.rearrange("(dk p) c -> p dk c", p=128)
    xs_r = xs.rearrange("(dk p) c -> p dk c", p=128)
    w1_r = w1t.rearrange("(dk p) i -> p dk i", p=128)
    w3_r = w3t.rearrange("(dk p) i -> p dk i", p=128)
    w2_r = w2t.rearrange("(mi p) d -> p mi d", p=128)
    ws1_r = ws1t.rearrange("(dk p) i -> p dk i", p=128)
    ws3_r = ws3t.rearrange("(dk p) i -> p dk i", p=128)
    ws2_r = ws2t.rearrange("(mi p) d -> p mi d", p=128)
    ye_r = ye.rearrange("(md p) c -> p md c", p=128)
    ys_r = ys.rearrange("(md p) c -> p md c", p=128)

    with tile.TileContext(nc) as tc:
        with tc.tile_pool(name="bias", bufs=1) as bpool, \
             tc.tile_pool(name="tmp", bufs=2) as tpool, \
             tc.tile_pool(name="yout", bufs=3) as ypool, \
             tc.tile_pool(name="ps", bufs=2, space="PSUM") as pspool:
            b1_sb = bpool.tile([128, NI], F32, tag="b1")
            nc.sync.dma_start(b1_sb[:], b1.rearrange("(mi p) -> p mi", p=128))
            b3_sb = bpool.tile([128, NI], F32, tag="b3")
            nc.sync.dma_start(b3_sb[:], b3.rearrange("(mi p) -> p mi", p=128))
            b2_sb = bpool.tile([128, ND], F32, tag="b2")
            nc.sync.dma_start(b2_sb[:], b2.rearrange("(md p) -> p md", p=128))
            bs1_sb = bpool.tile([128, NS], F32, tag="bs1")
            nc.sync.dma_start(bs1_sb[:], bs1.rearrange("(mi p) -> p mi", p=128))
            bs3_sb = bpool.tile([128, NS], F32, tag="bs3")
            nc.sync.dma_start(bs3_sb[:], bs3.rearrange("(mi p) -> p mi", p=128))
            bs2_sb = bpool.tile([128, ND], F32, tag="bs2")
            nc.sync.dma_start(bs2_sb[:], bs2.rearrange("(md p) -> p md", p=128))

            blocks = _blocks(C)
            nb = len(blocks)

            def load_xb(xpool, off, n):
                xb = []
                for dk in range(ND):
                    t = xpool.tile([128, n], F32R, tag=f"xb{dk}",
                                   name=f"xb{dk}", padded_shape=[128, BLK])
                    nc.sync.dma_start(t[:], xe_r[:, dk, off:off + n])
                    xb.append(t)
                return xb

            def mi_stage(w1_sb, w3_sb, xb, hb, n, width, nk, t1b, t3b, hoff=0):
                nm = width // 128
                for mi in range(nm):
                    ps1 = pspool.tile([128, n], F32, tag="ps1",
                                      padded_shape=[128, BLK])
                    ps3 = pspool.tile([128, n], F32, tag="ps3",
                                      padded_shape=[128, BLK])
                    for dk in range(nk):
                        nc.tensor.matmul(
                            ps1[:], w1_sb[dk][:, mi * 128:(mi + 1) * 128],
                            xb[dk][:, hoff:hoff + n],
                            start=(dk == 0), stop=(dk == nk - 1))
                    for dk in range(nk):
                        nc.tensor.matmul(
                            ps3[:], w3_sb[dk][:, mi * 128:(mi + 1) * 128],
                            xb[dk][:, hoff:hoff + n],
                            start=(dk == 0), stop=(dk == nk - 1))
                    hdt = hb[mi].dtype
                    tdt = BF16 if hdt == BF16 else F32
                    t1 = tpool.tile([128, n], tdt, tag=f"t1{tdt}",
                                    name="t1", padded_shape=[128, BLK])
                    nc.scalar.activation(t1[:], ps1[:], SILU,
                                         bias=t1b[:, mi:mi + 1])
                    t3 = tpool.tile([128, n], tdt, tag=f"t3{tdt}",
                                    name="t3", padded_shape=[128, BLK])
                    nc.scalar.activation(t3[:], ps3[:], IDENT,
                                         bias=t3b[:, mi:mi + 1])
                    nc.vector.tensor_mul(hb[mi][:], t1[:], t3[:])

            def md_stage(w2_sb, hb, out_r, off, n, nmi, b2b):
                for md in range(ND):
                    psy = pspool.tile([128, n], F32, tag="psy",
                                      padded_shape=[128, BLK])
                    for mi in range(nmi):
                        nc.tensor.matmul(
                            psy[:], w2_sb[mi][:, md * 128:(md + 1) * 128],
                            hb[mi][:],
                            start=(mi == 0), stop=(mi == nmi - 1))
                    yt = ypool.tile([128, n], F32, tag="yt",
                                    name="yt", padded_shape=[128, BLK])
                    nc.scalar.activation(yt[:], psy[:], IDENT,
                                         bias=b2b[:, md:md + 1])
                    nc.sync.dma_start(out_r[:, md, off:off + n], yt[:])

            # ---------- Phase 1: routed expert (fp32r, weights resident) ----
            with tc.tile_pool(name="wexp", bufs=1) as wpool, \
                 tc.tile_pool(name="xbp", bufs=2) as xpool, \
                 tc.tile_pool(name="hbp", bufs=1) as hpool:
                xb0 = load_xb(xpool, 0, blocks[0][1])
                w1_sb, w3_sb, w2_sb = [], [], []
                for dk in range(ND):
                    t = wpool.tile([128, INTER], F32R, tag=f"w1_{dk}")
                    nc.sync.dma_start(t[:], w1_r[:, dk, :])
                    w1_sb.append(t)
                for dk in range(ND):
                    t = wpool.tile([128, INTER], F32R, tag=f"w3_{dk}")
                    nc.sync.dma_start(t[:], w3_r[:, dk, :])
                    w3_sb.append(t)
                for mi in range(NI):
                    t = wpool.tile([128, DIM], F32R, tag=f"w2_{mi}")
                    nc.sync.dma_start(t[:], w2_r[:, mi, :])
                    w2_sb.append(t)

                for bi, (off, n) in enumerate(blocks):
                    xb = xb0 if bi == 0 else load_xb(xpool, off, n)
                    hb = [hpool.tile([128, n], F32R, tag=f"hb{mi}",
                                     name=f"hb{mi}", padded_shape=[128, BLK])
                          for mi in range(NI)]
                    mi_stage(w1_sb, w3_sb, xb, hb, n, INTER, ND, b1_sb, b3_sb)
                    md_stage(w2_sb, hb, ye_r, off, n, NI, b2_sb)

            # ---------- Phase 2: shared expert (bf16, weights resident) -----
            with tc.tile_pool(name="wsh", bufs=1) as wspool, \
                 tc.tile_pool(name="hsp", bufs=1) as hspool:
                xs_sb, ws1_sb, ws3_sb, ws2_sb = [], [], [], []
                for dk in range(ND):
                    t = wspool.tile([128, TOKS_SHARED], BF16, tag=f"xs{dk}")
                    nc.sync.dma_start(t[:], xs_r[:, dk, :])
                    xs_sb.append(t)
                    t = wspool.tile([128, SHARED_INTER], BF16, tag=f"ws1_{dk}")
                    nc.sync.dma_start(t[:], ws1_r[:, dk, :])
                    ws1_sb.append(t)
                for dk in range(ND):
                    t = wspool.tile([128, SHARED_INTER], BF16, tag=f"ws3_{dk}")
                    nc.sync.dma_start(t[:], ws3_r[:, dk, :])
                    ws3_sb.append(t)
                for mi in range(NS):
                    t = wspool.tile([128, DIM], BF16, tag=f"ws2_{mi}")
                    nc.sync.dma_start(t[:], ws2_r[:, mi, :])
                    ws2_sb.append(t)

                for (off, n) in _blocks(TOKS_SHARED):
                    hs = [hspool.tile([128, n], BF16, tag=f"hs{mi}",
                                      name=f"hs{mi}", padded_shape=[128, BLK])
                          for mi in range(NS)]
                    mi_stage(ws1_sb, ws3_sb, xs_sb, hs, n, SHARED_INTER, ND,
                             bs1_sb, bs3_sb, hoff=off)
                    md_stage(ws2_sb, hs, ys_r, off, n, NS, bs2_sb)

    nc.compile()
    return nc


def _gate_host(xt, gate_w, gate_b):
    """Softmax gate + top-2 routing, computed in float64 on the host."""
    logits = xt.astype(np.float64) @ gate_w.astype(np.float64).T \
        + gate_b.astype(np.float64)
    m = logits.max(axis=-1, keepdims=True)
    p = np.exp(logits - m)
    scores = p / p.sum(axis=-1, keepdims=True)
    order = np.argsort(-scores, axis=1, kind="stable")
    top_i = order[:, :TOPK]
    top_w = (np.take_along_axis(scores, top_i, axis=1)
             * ROUTE_SCALE).astype(np.float32)
    return top_i, top_w


def run(inputs, trace=False):
    x = np.ascontiguousarray(np.asarray(inputs["x"], dtype=np.float32))
    gate_w = np.asarray(inputs["gate_w"], dtype=np.float32)
    gate_b = np.asarray(inputs["gate_b"], dtype=np.float32)
    w1 = np.asarray(inputs["w1"], dtype=np.float32)
    b1 = np.asarray(inputs["b1"], dtype=np.float32)
    w3 = np.asarray(inputs["w3"], dtype=np.float32)
    b3 = np.asarray(inputs["b3"], dtype=np.float32)
    w2 = np.asarray(inputs["w2"], dtype=np.float32)
    b2 = np.asarray(inputs["b2"], dtype=np.float32)
    ws1 = np.asarray(inputs["ws1"], dtype=np.float32)
    bs1 = np.asarray(inputs["bs1"], dtype=np.float32)
    ws3 = np.asarray(inputs["ws3"], dtype=np.float32)
    bs3 = np.asarray(inputs["bs3"], dtype=np.float32)
    ws2 = np.asarray(inputs["ws2"], dtype=np.float32)
    bs2 = np.asarray(inputs["bs2"], dtype=np.float32)

    xt = x.reshape(T, DIM)
    top_i, top_w = _gate_host(xt, gate_w, gate_b)

    # Dispatch: token lists + gate weights per expert.
    idx, wgt = [], []
    for e in range(E):
        toks = np.nonzero((top_i == e).any(axis=1))[0]
        idx.append(toks)
        slot = (top_i[toks] == e)            # [n_e, TOPK], exactly one True/row
        wgt.append(top_w[toks][slot])

    cmax = max(len(i) for i in idx)
    C = max(256, -(-cmax // 256) * 256)

    bf16 = ml_dtypes.bfloat16
    ws1t = np.ascontiguousarray(ws1.T).astype(bf16)
    ws3t = np.ascontiguousarray(ws3.T).astype(bf16)
    ws2t = np.ascontiguousarray(ws2.T).astype(bf16)
    xt_bf = xt.astype(bf16)

    in_maps = []
    for e in range(E):
        xe = np.zeros((DIM, C), np.float32)
        xe[:, :len(idx[e])] = xt[idx[e]].T
        sl = slice(TOKS_SHARED * e, TOKS_SHARED * (e + 1))
        in_maps.append({
            "xe": xe,
            "xs": np.ascontiguousarray(xt_bf[sl].T),
            "w1t": np.ascontiguousarray(w1[e].T),
            "w3t": np.ascontiguousarray(w3[e].T),
            "w2t": np.ascontiguousarray(w2[e].T),
            "ws1t": ws1t, "ws3t": ws3t, "ws2t": ws2t,
            "b1": b1[e], "b3": b3[e], "b2": b2[e],
            "bs1": bs1, "bs3": bs3, "bs2": bs2,
        })

    if C not in _program_cache:
        _program_cache[C] = build_program(C)
    nc = _program_cache[C]

    res = bass_utils.run_bass_kernel_spmd(
        nc, in_maps, core_ids=list(range(N_CORES)), trace=trace)

    y = np.empty((T, DIM), np.float32)
    for e in range(E):
        sl = slice(TOKS_SHARED * e, TOKS_SHARED * (e + 1))
        y[sl] = res.results[e]["ys"].T
    for e in range(E):
        ye = res.results[e]["ye"]
        y[idx[e]] += ye[:, :len(idx[e])].T * wgt[e][:, None]
    return y.reshape(B, S, DIM), res


def kernel(**inputs) -> np.ndarray:
    out, _ = run(inputs, trace=False)
    return out


# revision 19
# speedup vs baseline: 1.0811x; 1.0264x over previous
"""MoE (DeepSeek-style) routed+shared expert forward on 8 TRN2 NeuronCores.

Strategy (expert-parallel, host-side dispatch):
  - Host computes the gate (softmax + top-2) in float64 and gathers each
    expert's routed tokens (this is the "all-to-all by routing index" --
    with full inputs on the host, the host does the dispatch).
  - Core e processes expert e's routed tokens (padded to a uniform
    capacity C) through the SwiGLU FFN, plus a 1/8 slice of all tokens
    through the replicated shared-expert MLP.
  - All activations/weights are fed transposed (features on SBUF
    partitions, tokens on the free dim) so the w1/w3 -> swiglu -> w2
    chain needs no on-chip transposes.
  - Matmuls use float32r (full-rate fp32 on the PE array).
  - Host scatters expert outputs back by routing index, scales by the
    gate weights, and adds the shared-expert output.
"""

import sys

if "/opt/trn_rl_repo" not in sys.path:
    sys.path.insert(0, "/opt/trn_rl_repo")

import ml_dtypes
import numpy as np

import concourse.bass as bass
import concourse.tile as tile
from concourse import bacc, mybir
from concourse import bass_utils

B, S, DIM = 4, 2048, 1024
T = B * S
INTER = 1024
E = 8
TOPK = 2
ROUTE_SCALE = 1.0
SHARED_INTER = 2048
N_CORES = 8
TOKS_SHARED = T // N_CORES  # shared-expert tokens per core
BLK = 512

F32 = mybir.dt.float32
F32R = mybir.dt.float32r
BF16 = mybir.dt.bfloat16
SILU = mybir.ActivationFunctionType.Silu
IDENT = mybir.ActivationFunctionType.Identity

_program_cache = {}


def _blocks(total):
    """Split `total` columns into blocks of 512, keeping every block
    >= 256 (fp32r matmuls drop to 1/4 rate below 256): a short tail
    is merged with the previous 512 and split into two halves."""
    assert total >= 256
    sizes = []
    rem = total
    while rem > 0:
        if rem >= BLK + 256 or rem <= BLK:
            n = min(BLK, rem)
            if n < 256:  # tail < 256: merge with previous block
                n2 = sizes.pop() + n
                h = (n2 // 2) & ~1
                sizes.extend([h, n2 - h])
                rem = 0
                continue
            sizes.append(n)
            rem -= n
        else:  # 513..767 left: split into two even halves >= 256
            h = (rem // 2) & ~1
            sizes.extend([h, rem - h])
            rem = 0
    out, o = [], 0
    for n in sizes:
        out.append((o, n))
        o += n
    return out


def build_program(C):
    """Build the per-core SPMD Bass program for routed capacity C.

    Phase 1 (routed expert): fp32r matmuls, w1/w3/w2 resident in SBUF.
    Phase 2 (shared expert): bf16 matmuls, ws1/ws3/ws2 resident in SBUF,
    tokens processed in two halves of 512. Each weight/activation chunk
    is a separate tile so matmuls depend only on the chunk they read;
    DMA issue order matches PE consumption order.
    """
    nc = bacc.Bacc("TRN2", target_bir_lowering=False, debug=False,
                   num_devices=N_CORES)

    def din(name, shape, dt=F32):
        return nc.dram_tensor(name, shape, dt, kind="ExternalInput").ap()

    def dout(name, shape):
        return nc.dram_tensor(name, shape, F32, kind="ExternalOutput").ap()

    xe = din("xe", (DIM, C), F32R)            # routed tokens, transposed
    xs = din("xs", (DIM, TOKS_SHARED), BF16)  # shared-token slice, transposed
    w1t = din("w1t", (DIM, INTER), F32R)      # w1[e].T
    w3t = din("w3t", (DIM, INTER), F32R)
    w2t = din("w2t", (INTER, DIM), F32R)      # w2[e].T
    ws1t = din("ws1t", (DIM, SHARED_INTER), BF16)
    ws3t = din("ws3t", (DIM, SHARED_INTER), BF16)
    ws2t = din("ws2t", (SHARED_INTER, DIM), BF16)
    b1 = din("b1", (INTER,))
    b3 = din("b3", (INTER,))
    b2 = din("b2", (DIM,))
    bs1 = din("bs1", (SHARED_INTER,))
    bs3 = din("bs3", (SHARED_INTER,))
    bs2 = din("bs2", (DIM,))
    ye = dout("ye", (DIM, C))
    ys = dout("ys", (DIM, TOKS_SHARED))

    ND = DIM // 128           # 8 k-tiles over DIM
    NI = INTER // 128         # 8 tiles over INTER
    NS = SHARED_INTER // 128  # 16 tiles over SHARED_INTER

    xe_r = xe.rearrange("(dk p) c -> p dk c", p=128)
    xs_r = xs.rearrange("(dk p) c -> p dk c", p=128)
    w1_r = w1t.rearrange("(dk p) i -> p dk i", p=128)
    w3_r = w3t.rearrange("(dk p) i -> p dk i", p=128)
    w2_r = w2t.rearrange("(mi p) d -> p mi d", p=128)
    ws1_r = ws1t.rearrange("(dk p) i -> p dk i", p=128)
    ws3_r = ws3t.rearrange("(dk p) i -> p dk i", p=128)
    ws2_r = ws2t.rearrange("(mi p) d -> p mi d", p=128)
    ye_r = ye.rearrange("(md p) c -> p md c", p=128)
    ys_r = ys.rearrange("(md p) c -> p md c", p=128)

    with tile.TileContext(nc) as tc:
        with tc.tile_pool(name="bias", bufs=1) as bpool, \
             tc.tile_pool(name="tmp", bufs=2) as tpool, \
             tc.tile_pool(name="yout", bufs=3) as ypool, \
             tc.tile_pool(name="ps", bufs=2, space="PSUM") as pspool:
            b1_sb = bpool.tile([128, NI], F32, tag="b1")
            nc.sync.dma_start(b1_sb[:], b1.rearrange("(mi p) -> p mi", p=128))
            b3_sb = bpool.tile([128, NI], F32, tag="b3")
            nc.sync.dma_start(b3_sb[:], b3.rearrange("(mi p) -> p mi", p=128))
            b2_sb = bpool.tile([128, ND], F32, tag="b2")
            nc.sync.dma_start(b2_sb[:], b2.rearrange("(md p) -> p md", p=128))
            bs1_sb = bpool.tile([128, NS], F32, tag="bs1")
            nc.sync.dma_start(bs1_sb[:], bs1.rearrange("(mi p) -> p mi", p=128))
            bs3_sb = bpool.tile([128, NS], F32, tag="bs3")
            nc.sync.dma_start(bs3_sb[:], bs3.rearrange("(mi p) -> p mi", p=128))
            bs2_sb = bpool.tile([128, ND], F32, tag="bs2")
            nc.sync.dma_start(bs2_sb[:], bs2.rearrange("(md p) -> p md", p=128))

            blocks = _blocks(C)
            nb = len(blocks)

            def load_xb(xpool, off, n):
                xb = []
                for dk in range(ND):
                    t = xpool.tile([128, n], F32R, tag=f"xb{dk}",
                                   name=f"xb{dk}", padded_shape=[128, BLK])
                    nc.sync.dma_start(t[:], xe_r[:, dk, off:off + n])
                    xb.append(t)
                return xb

            def mi_stage(w1_sb, w3_sb, xb, hb, n, width, nk, t1b, t3b, hoff=0):
                nm = width // 128
                for mi in range(nm):
                    ps1 = pspool.tile([128, n], F32, tag="ps1",
                                      padded_shape=[128, BLK])
                    ps3 = pspool.tile([128, n], F32, tag="ps3",
                                      padded_shape=[128, BLK])
                    for dk in range(nk):
                        nc.tensor.matmul(
                            ps1[:], w1_sb[dk][:, mi * 128:(mi + 1) * 128],
                            xb[dk][:, hoff:hoff + n],
                            start=(dk == 0), stop=(dk == nk - 1))
                    for dk in range(nk):
                        nc.tensor.matmul(
                            ps3[:], w3_sb[dk][:, mi * 128:(mi + 1) * 128],
                            xb[dk][:, hoff:hoff + n],
                            start=(dk == 0), stop=(dk == nk - 1))
                    hdt = hb[mi].dtype
                    tdt = BF16 if hdt == BF16 else F32
                    t1 = tpool.tile([128, n], tdt, tag=f"t1{tdt}",
                                    name="t1", padded_shape=[128, BLK])
                    nc.scalar.activation(t1[:], ps1[:], SILU,
                                         bias=t1b[:, mi:mi + 1])
                    t3 = tpool.tile([128, n], tdt, tag=f"t3{tdt}",
                                    name="t3", padded_shape=[128, BLK])
                    nc.scalar.activation(t3[:], ps3[:], IDENT,
                                         bias=t3b[:, mi:mi + 1])
                    nc.vector.tensor_mul(hb[mi][:], t1[:], t3[:])

            def md_stage(w2_sb, hb, out_r, off, n, nmi, b2b):
                for md in range(ND):
                    psy = pspool.tile([128, n], F32, tag="psy",
                                      padded_shape=[128, BLK])
                    for mi in range(nmi):
                        nc.tensor.matmul(
                            psy[:], w2_sb[mi][:, md * 128:(md + 1) * 128],
                            hb[mi][:],
                            start=(mi == 0), stop=(mi == nmi - 1))
                    yt = ypool.tile([128, n], F32, tag="yt",
                                    name="yt", padded_shape=[128, BLK])
                    nc.scalar.activation(yt[:], psy[:], IDENT,
                                         bias=b2b[:, md:md + 1])
                    nc.sync.dma_start(out_r[:, md, off:off + n], yt[:])

            # ---------- Phase 1: routed expert (fp32r, weights resident) ----
            with tc.tile_pool(name="wexp", bufs=1) as wpool, \
                 tc.tile_pool(name="xbp", bufs=2) as xpool, \
                 tc.tile_pool(name="hbp", bufs=1) as hpool:
                xb0 = load_xb(xpool, 0, blocks[0][1])
                w1_sb, w3_sb, w2_sb = [], [], []
                for dk in range(ND):
                    t = wpool.tile([128, INTER], F32R, tag=f"w1_{dk}")
                    nc.sync.dma_start(t[:], w1_r[:, dk, :])
                    w1_sb.append(t)
                for dk in range(ND):
                    t = wpool.tile([128, INTER], F32R, tag=f"w3_{dk}")
                    nc.sync.dma_start(t[:], w3_r[:, dk, :])
                    w3_sb.append(t)
                for mi in range(NI):
                    t = wpool.tile([128, DIM], F32R, tag=f"w2_{mi}")
                    nc.sync.dma_start(t[:], w2_r[:, mi, :])
                    w2_sb.append(t)

                for bi, (off, n) in enumerate(blocks):
                    xb = xb0 if bi == 0 else load_xb(xpool, off, n)
                    hb = [hpool.tile([128, n], F32R, tag=f"hb{mi}",
                                     name=f"hb{mi}", padded_shape=[128, BLK])
                          for mi in range(NI)]
                    if bi == 0:
                        t1w = []
                        for mi in range(NI):
                            ps1 = pspool.tile([128, n], F32, tag="ps1",
                                              padded_shape=[128, BLK])
                            for dk in range(ND):
                                nc.tensor.matmul(
                                    ps1[:],
                                    w1_sb[dk][:, mi * 128:(mi + 1) * 128],
                                    xb[dk][:],
                                    start=(dk == 0), stop=(dk == ND - 1))
                            t1 = tpool.tile([128, n], F32, tag=f"t1w{mi}",
                                            name=f"t1w{mi}", bufs=1,
                                            padded_shape=[128, BLK])
                            nc.scalar.activation(t1[:], ps1[:], SILU,
                                                 bias=b1_sb[:, mi:mi + 1])
                            t1w.append(t1)
                        for mi in range(NI):
                            ps3 = pspool.tile([128, n], F32, tag="ps3",
                                              padded_shape=[128, BLK])
                            for dk in range(ND):
                                nc.tensor.matmul(
                                    ps3[:],
                                    w3_sb[dk][:, mi * 128:(mi + 1) * 128],
                                    xb[dk][:],
                                    start=(dk == 0), stop=(dk == ND - 1))
                            t3 = tpool.tile([128, n], F32, tag="t3dt.float32",
                                            name="t3", padded_shape=[128, BLK])
                            nc.scalar.activation(t3[:], ps3[:], IDENT,
                                                 bias=b3_sb[:, mi:mi + 1])
                            nc.vector.tensor_mul(hb[mi][:], t1w[mi][:], t3[:])
                    else:
                        mi_stage(w1_sb, w3_sb, xb, hb, n, INTER, ND,
                                 b1_sb, b3_sb)
                    md_stage(w2_sb, hb, ye_r, off, n, NI, b2_sb)

            # ---------- Phase 2: shared expert (bf16, weights resident) -----
            with tc.tile_pool(name="wsh", bufs=1) as wspool, \
                 tc.tile_pool(name="hsp", bufs=1) as hspool:
                xs_sb, ws1_sb, ws3_sb, ws2_sb = [], [], [], []
                for dk in range(ND):
                    t = wspool.tile([128, TOKS_SHARED], BF16, tag=f"xs{dk}")
                    nc.sync.dma_start(t[:], xs_r[:, dk, :])
                    xs_sb.append(t)
                    t = wspool.tile([128, SHARED_INTER], BF16, tag=f"ws1_{dk}")
                    nc.sync.dma_start(t[:], ws1_r[:, dk, :])
                    ws1_sb.append(t)
                    t = wspool.tile([128, SHARED_INTER], BF16, tag=f"ws3_{dk}")
                    nc.sync.dma_start(t[:], ws3_r[:, dk, :])
                    ws3_sb.append(t)
                for mi in range(NS):
                    t = wspool.tile([128, DIM], BF16, tag=f"ws2_{mi}")
                    nc.sync.dma_start(t[:], ws2_r[:, mi, :])
                    ws2_sb.append(t)

                for (off, n) in _blocks(TOKS_SHARED):
                    hs = [hspool.tile([128, n], BF16, tag=f"hs{mi}",
                                      name=f"hs{mi}", padded_shape=[128, BLK])
                          for mi in range(NS)]
                    mi_stage(ws1_sb, ws3_sb, xs_sb, hs, n, SHARED_INTER, ND,
                             bs1_sb, bs3_sb, hoff=off)
                    md_stage(ws2_sb, hs, ys_r, off, n, NS, bs2_sb)

    nc.compile()
    return nc


def _gate_host(xt, gate_w, gate_b):
    """Softmax gate + top-2 routing, computed in float64 on the host."""
    logits = xt.astype(np.float64) @ gate_w.astype(np.float64).T \
        + gate_b.astype(np.float64)
    m = logits.max(axis=-1, keepdims=True)
    p = np.exp(logits - m)
    scores = p / p.sum(axis=-1, keepdims=True)
    order = np.argsort(-scores, axis=1, kind="stable")
    top_i = order[:, :TOPK]
    top_w = (np.take_along_axis(scores, top_i, axis=1)
             * ROUTE_SCALE).astype(np.float32)
    return top_i, top_w


def run(inputs, trace=False):
    x = np.ascontiguousarray(np.asarray(inputs["x"], dtype=np.float32))
    gate_w = np.asarray(inputs["gate_w"], dtype=np.float32)
    gate_b = np.asarray(inputs["gate_b"], dtype=np.float32)
    w1 = np.asarray(inputs["w1"], dtype=np.float32)
    b1 = np.asarray(inputs["b1"], dtype=np.float32)
    w3 = np.asarray(inputs["w3"], dtype=np.float32)
    b3 = np.asarray(inputs["b3"], dtype=np.float32)
    w2 = np.asarray(inputs["w2"], dtype=np.float32)
    b2 = np.asarray(inputs["b2"], dtype=np.float32)
    ws1 = np.asarray(inputs["ws1"], dtype=np.float32)
    bs1 = np.asarray(inputs["bs1"], dtype=np.float32)
    ws3 = np.asarray(inputs["ws3"], dtype=np.float32)
    bs3 = np.asarray(inputs["bs3"], dtype=np.float32)
    ws2 = np.asarray(inputs["ws2"], dtype=np.float32)
    bs2 = np.asarray(inputs["bs2"], dtype=np.float32)

    xt = x.reshape(T, DIM)
    top_i, top_w = _gate_host(xt, gate_w, gate_b)

    # Dispatch: token lists + gate weights per expert.
    idx, wgt = [], []
    for e in range(E):
        toks = np.nonzero((top_i == e).any(axis=1))[0]
        idx.append(toks)
        slot = (top_i[toks] == e)            # [n_e, TOPK], exactly one True/row
        wgt.append(top_w[toks][slot])

    cmax = max(len(i) for i in idx)
    C = max(256, cmax + (cmax & 1))   # fp32r matmul needs an even free dim

    bf16 = ml_dtypes.bfloat16
    ws1t = np.ascontiguousarray(ws1.T).astype(bf16)
    ws3t = np.ascontiguousarray(ws3.T).astype(bf16)
    ws2t = np.ascontiguousarray(ws2.T).astype(bf16)
    xt_bf = xt.astype(bf16)

    in_maps = []
    for e in range(E):
        xe = np.zeros((DIM, C), np.float32)
        xe[:, :len(idx[e])] = xt[idx[e]].T
        sl = slice(TOKS_SHARED * e, TOKS_SHARED * (e + 1))
        in_maps.append({
            "xe": xe,
            "xs": np.ascontiguousarray(xt_bf[sl].T),
            "w1t": np.ascontiguousarray(w1[e].T),
            "w3t": np.ascontiguousarray(w3[e].T),
            "w2t": np.ascontiguousarray(w2[e].T),
            "ws1t": ws1t, "ws3t": ws3t, "ws2t": ws2t,
            "b1": b1[e], "b3": b3[e], "b2": b2[e],
            "bs1": bs1, "bs3": bs3, "bs2": bs2,
        })

    if C not in _program_cache:
        _program_cache[C] = build_program(C)
    nc = _program_cache[C]

    res = bass_utils.run_bass_kernel_spmd(
        nc, in_maps, core_ids=list(range(N_CORES)), trace=trace)

    y = np.empty((T, DIM), np.float32)
    for e in range(E):
        sl = slice(TOKS_SHARED * e, TOKS_SHARED * (e + 1))
        y[sl] = res.results[e]["ys"].T
    for e in range(E):
        ye = res.results[e]["ye"]
        y[idx[e]] += ye[:, :len(idx[e])].T * wgt[e][:, None]
    return y.reshape(B, S, DIM), res


def kernel(**inputs) -> np.ndarray:
    out, _ = run(inputs, trace=False)
    return out
